# revision 1
# baseline (speedup 1.0000x reference)
"""Multi-head attention TRN2 kernel (8 NeuronCores, SPMD).

Problem: B=2, N=2048, D=1024, H=16 heads of dim 64, fp32, per-(b,h)
key-length masking (valid_len, length 32).

Sharding: batch*heads across 8 cores — core c handles batch b=c//4 and 4
heads ("slots").  Per core:
  Q^T/K^T = Wslice^T @ x^T   (heads on partitions, positions on free)
  V       = x^T-tiles as lhsT, Wv as rhs  (positions on partitions)
  S^T     = K^T.T @ Q^T  per head (row-packed K=64 pairs)
  P^T     = exp(S^T / 8) (ScalarE, fused scale)
  heads^T = [V|1].T @ P^T  accumulated over key tiles (ones column gives
            the softmax denominator as row 64; masking is folded into V
            by zeroing rows >= valid_len via a per-partition mask scale)
  normalize via DVE reciprocal + one Newton step, PE K=1 broadcast
  out_partial = heads^T.T @ Wo_slice  -> (2048, 1024) per core
Host sums the 4 per-core partials of each batch element (the unshard for
the row-sharded Wo) and gathers.

All matmuls run as float32r (TF32-like, ~1e-4 rel err, full PE rate).
The per-head key loop is specialized at build time to
ceil(max_vl_in_slot/128) tiles; exact masking comes from the mask scale.
"""
import sys
import numpy as np
from contextlib import ExitStack

sys.path.insert(0, "/opt/trn_rl_repo")

import concourse.bass as bass  # noqa: E402
from concourse import bacc, mybir  # noqa: E402
import concourse.tile as tile  # noqa: E402
from concourse.bass_utils import run_bass_kernel_spmd  # noqa: E402

F32 = mybir.dt.float32
F32R = mybir.dt.float32r
BF16 = mybir.dt.bfloat16
AF = mybir.ActivationFunctionType

B, N, D, H = 2, 2048, 1024, 16
DH = 64
HPC = 4          # heads (slots) per core
NCORES = 8
QC = 512         # q chunk (matmul free dim)
NQT = N // 128   # 16 q tiles
NKT = N // 128   # 16 k tiles
NDC = D // 128   # 8 contraction chunks

LAST_RESULTS = None  # BassKernelResults of the most recent run (for tooling)


def _build_program(trips):
    """trips: 4 ints — k-tile count per slot (uniform across cores)."""
    nc = bacc.Bacc("TRN2", target_bir_lowering=False, debug=False,
                   num_devices=NCORES)

    xTq = nc.dram_tensor("xTq", [D, N], F32R, kind="ExternalInput")
    xTk = nc.dram_tensor("xTk", [D, N], F32R, kind="ExternalInput")
    xTv = nc.dram_tensor("xTv", [D, N], F32R, kind="ExternalInput")
    wq = nc.dram_tensor("wq", [128, NDC * 256], F32R, kind="ExternalInput")
    wk = nc.dram_tensor("wk", [128, NDC * 256], F32R, kind="ExternalInput")
    wv = nc.dram_tensor("wv", [128, NDC * 256], F32R, kind="ExternalInput")
    wo = nc.dram_tensor("wo", [256, D], F32R, kind="ExternalInput")
    vmask = nc.dram_tensor("vmask", [128, HPC * NKT], F32, kind="ExternalInput")
    out = nc.dram_tensor("out", [N, D], F32, kind="ExternalOutput")

    with tile.TileContext(nc) as tc:
        with ExitStack() as ctx:
            wpool = ctx.enter_context(tc.tile_pool(name="wpool", bufs=1))
            xpool = ctx.enter_context(tc.tile_pool(name="xpool", bufs=3))
            qkpool = ctx.enter_context(tc.tile_pool(name="qkpool", bufs=1))
            v1pool = ctx.enter_context(tc.tile_pool(name="v1pool", bufs=1))
            ptpool = ctx.enter_context(tc.tile_pool(name="ptpool", bufs=4))
            nrmpool = ctx.enter_context(tc.tile_pool(name="nrmpool", bufs=2))
            pbpool = ctx.enter_context(tc.tile_pool(name="pbpool", bufs=1))
            opool = ctx.enter_context(tc.tile_pool(name="opool", bufs=3))

            t_wq = wpool.tile([128, NDC * 256], F32R, tag="wq")
            t_wk = wpool.tile([128, NDC * 256], F32R, tag="wk")
            t_wv = wpool.tile([128, NDC * 256], F32R, tag="wv")
            nc.sync.dma_start(t_wq[:], wq[:])
            nc.sync.dma_start(t_wk[:], wk[:])
            nc.sync.dma_start(t_wv[:], wv[:])
            t_wo = [wpool.tile([128, D], F32R, tag=f"wo{p}", name=f"t_wo{p}") for p in range(2)]
            nc.sync.dma_start(t_wo[0][:], wo[0:128, :])
            nc.sync.dma_start(t_wo[1][:], wo[128:256, :])
            t_vm = wpool.tile([128, HPC * NKT], F32, tag="vm")
            nc.sync.dma_start(t_vm[:], vmask[:])

            # Q^T / K^T: [128 dims (2 slots), N] per slot-pair
            t_qT = [qkpool.tile([128, N], F32R, tag=f"qT{p}", name=f"t_qT{p}") for p in range(2)]
            t_kT = [qkpool.tile([128, N], F32R, tag=f"kT{p}", name=f"t_kT{p}") for p in range(2)]
            # V1 per slot: NKT tiles of [128, 65] ([V | ones-masked])
            t_v1 = [v1pool.tile([128, 65 * trips[j]], F32R, tag=f"v1_{j}", name=f"t_v1_{j}")
                    for j in range(HPC)]
            # normalized heads^T per slot pair: [128 dims, N]
            t_pb = [pbpool.tile([128, N], F32R, tag=f"pb{p}", name=f"t_pb{p}") for p in range(2)]

            # ---- phase 1: projections ----
            with tc.tile_pool(name="pp", bufs=8, space="PSUM") as pp:
                for xin, wsb, dsts in ((xTq, t_wq, t_qT), (xTk, t_wk, t_kT)):
                    accs = [pp.tile([128, QC], F32, tag="acc", name=f"acc_{i}")
                            for i in range(2 * (N // QC))]
                    for c in range(NDC):
                        xt = xpool.tile([128, N], F32R, tag="xt")
                        nc.sync.dma_start(xt[:], xin[c * 128:(c + 1) * 128, :])
                        for m in range(2):
                            for q in range(N // QC):
                                nc.tensor.matmul(
                                    accs[m * (N // QC) + q][:],
                                    wsb[:, c * 256 + m * 128: c * 256 + (m + 1) * 128],
                                    xt[:, q * QC:(q + 1) * QC],
                                    start=(c == 0), stop=(c == NDC - 1))
                    for m in range(2):
                        for q in range(N // QC):
                            i = m * (N // QC) + q
                            dst = dsts[m][:, q * QC:(q + 1) * QC]
                            if i % 2 == 0:
                                nc.scalar.activation(dst, accs[i][:], AF.Copy)
                            else:
                                with nc.allow_low_precision(reason="f32r 4B"):
                                    nc.vector.tensor_copy(dst, accs[i][:])
                # V projection: two half-column passes of 8 k-tiles
                for g in range(2):
                    accs = [pp.tile([128, 256], F32, tag="acc", name=f"accv_{i}") for i in range(8)]
                    for c in range(NDC):
                        xt = xpool.tile([128, 1024], F32R, tag="xtv")
                        nc.sync.dma_start(
                            xt[:], xTv[c * 128:(c + 1) * 128,
                                       g * 1024:(g + 1) * 1024])
                        for kt8 in range(8):
                            nc.tensor.matmul(
                                accs[kt8][:],
                                xt[:, kt8 * 128:(kt8 + 1) * 128],
                                wsb_v_rhs(t_wv, c),
                                start=(c == 0), stop=(c == NDC - 1))
                    for kt8 in range(8):
                        t = g * 8 + kt8
                        for j in range(HPC):
                            if t >= trips[j]:
                                continue
                            mask_col = t_vm[:, j * NKT + t: j * NKT + t + 1]
                            # V columns scaled by mask (zero rows >= vl)
                            nc.scalar.activation(
                                t_v1[j][:, t * 65: t * 65 + 64],
                                accs[kt8][:, j * 64:(j + 1) * 64],
                                AF.Copy, scale=mask_col)
                            # ones column = mask itself
                            nc.vector.tensor_copy(
                                t_v1[j][:, t * 65 + 64: t * 65 + 65], mask_col)

            # ---- phase 2+3: attention with fused output projection ----
            with tc.tile_pool(name="ap", bufs=1, space="PSUM") as ap:
                for q in range(N // QC):
                    qs = slice(q * QC, (q + 1) * QC)
                    for p in range(2):
                        ja, jb = 2 * p, 2 * p + 1
                        acc_a = ap.tile([65, QC], F32, tag="acc2", bufs=4,
                                        name="acc_a")
                        acc_b = ap.tile([65, QC], F32, tag="acc2", bufs=4,
                                        name="acc_b")
                        for t in range(trips[ja]):
                            ks = slice(t * 128, (t + 1) * 128)
                            sT_a = ap.tile([128, QC], F32, tag="sT", bufs=2)
                            nc.tensor.matmul(sT_a[:], t_kT[p][0:64, ks],
                                             t_qT[p][0:64, qs],
                                             start=True, stop=True)
                            pT_a = ptpool.tile([128, QC], F32R, tag="pT")
                            nc.scalar.activation(pT_a[:], sT_a[:], AF.Exp,
                                                 scale=0.125)
                            nc.tensor.matmul(
                                acc_a[:], t_v1[ja][:, t * 65:(t + 1) * 65],
                                pT_a[:], start=(t == 0),
                                stop=(t == trips[ja] - 1))
                            if t < trips[jb]:
                                sT_b = ap.tile([128, QC], F32, tag="sT", bufs=2)
                                nc.tensor.matmul(sT_b[:], t_kT[p][64:128, ks],
                                                 t_qT[p][64:128, qs],
                                                 start=True, stop=True)
                                pT_b = ptpool.tile([128, QC], F32R, tag="pT")
                                nc.scalar.activation(pT_b[:], sT_b[:], AF.Exp,
                                                     scale=0.125)
                                nc.tensor.matmul(
                                    acc_b[:], t_v1[jb][:, t * 65:(t + 1) * 65],
                                    pT_b[:], start=(t == 0),
                                    stop=(t == trips[jb] - 1))
                        # normalize both slots of the pair (bcast on GpSimd)
                        for e, acc in ((0, acc_a), (1, acc_b)):
                            r0 = nrmpool.tile([1, QC], F32, tag="r0")
                            nc.vector.reciprocal(r0[:], acc[64:65, :])
                            # Newton: r1 = r0 * (2 - d*r0)
                            t1 = nrmpool.tile([1, QC], F32, tag="t1")
                            nc.vector.tensor_mul(t1[:], acc[64:65, :], r0[:])
                            t2 = nrmpool.tile([1, QC], F32, tag="t2")
                            nc.vector.tensor_scalar(
                                t2[:], t1[:], -1.0, 2.0,
                                mybir.AluOpType.mult, mybir.AluOpType.add)
                            r1 = nrmpool.tile([1, QC], F32, tag="r1")
                            nc.vector.tensor_mul(r1[:], r0[:], t2[:])
                            bc_sb = nrmpool.tile([64, QC], F32, tag="bc_sb")
                            nc.gpsimd.partition_broadcast(bc_sb[:], r1[:])
                            with nc.allow_low_precision(reason="f32r 4B"):
                                nc.vector.tensor_mul(
                                    t_pb[p][e * 64:(e + 1) * 64, qs],
                                    acc[0:64, :], bc_sb[:])
                    # output projection for the 4 q-tiles of this chunk
                    for qt in range(q * (QC // 128), (q + 1) * (QC // 128)):
                        ts = slice(qt * 128, (qt + 1) * 128)
                        stage = opool.tile([128, D], F32, tag="ostage")
                        for ch in range(2):
                            o_ps = ap.tile([128, 512], F32, tag="o", bufs=2)
                            for p in range(2):
                                nc.tensor.matmul(
                                    o_ps[:], t_pb[p][:, ts],
                                    t_wo[p][:, ch * 512:(ch + 1) * 512],
                                    start=(p == 0), stop=(p == 1))
                            nc.vector.tensor_copy(
                                stage[:, ch * 512:(ch + 1) * 512], o_ps[:])
                        nc.sync.dma_start(out[ts, :], stage[:])

    nc.finalize()
    return nc


def wsb_v_rhs(t_wv, c):
    return t_wv[:, c * 256:(c + 1) * 256]


def kernel(queries, keys, values, valid_len, Wq, Wk, Wv, Wo):
    global LAST_RESULTS
    queries = np.asarray(queries, dtype=np.float32)
    keys = np.asarray(keys, dtype=np.float32)
    values = np.asarray(values, dtype=np.float32)
    Wq = np.asarray(Wq, dtype=np.float32)
    Wk = np.asarray(Wk, dtype=np.float32)
    Wv = np.asarray(Wv, dtype=np.float32)
    Wo = np.asarray(Wo, dtype=np.float32)
    vl = np.asarray(valid_len).astype(np.int64).reshape(B * H)

    # rank-aligned slot assignment: per batch, heads sorted by vl desc;
    # slot j of the 4 cores of that batch takes ranks 4j..4j+3
    order = {}
    for b in range(B):
        idx = (np.argsort(-vl[b * H:(b + 1) * H], kind="stable") + b * H)
        for cg in range(4):
            order[b * 4 + cg] = [int(idx[4 * j + cg]) for j in range(HPC)]
    trips = []
    for j in range(HPC):
        m = max(int(-(-vl[order[c][j]] // 128)) for c in range(NCORES))
        trips.append(max(1, min(NKT, m)))

    nc = _build_program(tuple(trips))

    in_maps = []
    for c in range(NCORES):
        b = c // 4
        heads = order[c]
        cols = np.concatenate(
            [np.arange((h - b * H) * DH, (h - b * H + 1) * DH) for h in heads])

        def wlayout(w):
            return np.ascontiguousarray(
                w[:, cols].reshape(NDC, 128, 256).transpose(1, 0, 2)
                .reshape(128, NDC * 256))

        vm = np.zeros((128, HPC * NKT), np.float32)
        for j, h in enumerate(heads):
            keep = (np.arange(N) < vl[h]).astype(np.float32)
            vm[:, j * NKT:(j + 1) * NKT] = keep.reshape(NKT, 128).T

        in_maps.append({
            "xTq": np.ascontiguousarray(queries[b].T),
            "xTk": np.ascontiguousarray(keys[b].T),
            "xTv": np.ascontiguousarray(values[b].T),
            "wq": wlayout(Wq),
            "wk": wlayout(Wk),
            "wv": wlayout(Wv),
            "wo": np.ascontiguousarray(Wo[cols, :]),
            "vmask": vm,
        })

    LAST_RESULTS = run_bass_kernel_spmd(nc, in_maps, list(range(NCORES)))
    res = LAST_RESULTS.results

    out = np.zeros((B, N, D), np.float64)
    for c in range(NCORES):
        out[c // 4] += res[c]["out"].astype(np.float64)
    return out.astype(np.float32)



# revision 12
# speedup vs baseline: 1.7593x; 1.7593x over previous
"""Multi-head attention TRN2 kernel (8 NeuronCores, SPMD).

Problem: B=2, N=2048, D=1024, H=16 heads of dim 64, fp32 in/out, per-(b,h)
key-length masking (valid_len, length 32).

Sharding: batch*heads across 8 cores - core c handles batch b=c//4 and 4
heads ("slots"), rank-aligned by valid_len so the per-slot key-tile trip
counts (uniform across cores, specialized at build time) are minimal.

v2 design (HAM-warm, ACT-paced):
  - All matmul operands bf16 (hosts converts; PSUM accumulation stays fp32).
    Halves DMA + SBUF traffic, enables FWL weight loads. rel-err ~5e-3.
  - S^T per slot-pair computed as two K=64 matmuls emitted back-to-back:
    tile_position auto-derives (0,0)/(64,0) from base partitions, so the
    two matmuls run CONCURRENTLY in different PE row-groups.
  - The two S^T tiles land in one [128,1024] 2-bank PSUM tile; ONE fused
    Exp activation (scale=1/8) covers both slots -> bf16 pT.
  - Softmax denominator via ones-column in V1 (mask folded into V rows).
  - Normalization: DVE reciprocal_approx_fast (no Newton, ~18 bits) +
    GpSimd partition_broadcast + DVE multiply -> bf16 headsT.
  - Q projection is per-chunk and software-pipelined into the previous
    chunk's attention ladder; output projection of chunk q is interleaved
    into ladder(q+1, p0). PE never idles >1us -> HAM stays at K=8/8.
  - PSUM budget (8 banks): sT 2x2 + acc 2 + mix(outproj/qproj) 2.
"""
import sys
import numpy as np
from contextlib import ExitStack

sys.path.insert(0, "/opt/trn_rl_repo")

import ml_dtypes  # noqa: E402
import concourse.bass as bass  # noqa: E402
from concourse import bacc, mybir  # noqa: E402
import concourse.tile as tile  # noqa: E402
from concourse.bass_utils import run_bass_kernel_spmd  # noqa: E402

F32 = mybir.dt.float32
BF16 = mybir.dt.bfloat16
AF = mybir.ActivationFunctionType
BFNP = ml_dtypes.bfloat16

B, N, D, H = 2, 2048, 1024, 16
DH = 64
HPC = 4          # heads (slots) per core
NCORES = 8
QC = 512         # q chunk (matmul free dim)
NQT = N // 128   # 16 q tiles
NKT = N // 128   # 16 k tiles
NDC = D // 128   # 8 contraction chunks
NCH = N // QC    # 4 q chunks

LAST_RESULTS = None  # BassKernelResults of the most recent run (for tooling)
DEBUG_DUMP = False   # add DRAM dumps of intermediates (debugging only)


def _build_program(trips):
    """trips: 4 ints - k-tile count per slot (uniform across cores)."""
    nc = bacc.Bacc("TRN2", target_bir_lowering=False, debug=False,
                   num_devices=NCORES)

    xTq = nc.dram_tensor("xTq", [D, N], BF16, kind="ExternalInput")
    xTk = nc.dram_tensor("xTk", [D, N], BF16, kind="ExternalInput")
    xTv = nc.dram_tensor("xTv", [D, N], BF16, kind="ExternalInput")
    wq = nc.dram_tensor("wq", [128, NDC * 256], BF16, kind="ExternalInput")
    wk = nc.dram_tensor("wk", [128, NDC * 256], BF16, kind="ExternalInput")
    wv = nc.dram_tensor("wv", [128, NDC * 256], BF16, kind="ExternalInput")
    wo = nc.dram_tensor("wo", [256, D], BF16, kind="ExternalInput")
    vmask = nc.dram_tensor("vmask", [128, HPC * NKT], F32, kind="ExternalInput")
    out = nc.dram_tensor("out", [N, D], F32, kind="ExternalOutput")
    dbg = {}
    if DEBUG_DUMP:
        for p in range(2):
            dbg[f"qT{p}"] = nc.dram_tensor(f"d_qT{p}", [128, N], BF16,
                                           kind="ExternalOutput")
            dbg[f"kT{p}"] = nc.dram_tensor(f"d_kT{p}", [128, N], BF16,
                                           kind="ExternalOutput")
            dbg[f"pb{p}"] = nc.dram_tensor(f"d_pb{p}", [128, N], BF16,
                                           kind="ExternalOutput")
        for j in range(HPC):
            dbg[f"v1_{j}"] = nc.dram_tensor(f"d_v1_{j}", [128, 65 * trips[j]],
                                            BF16, kind="ExternalOutput")
        dbg["sT00"] = nc.dram_tensor("d_sT00", [128, 1024], F32,
                                     kind="ExternalOutput")
        dbg["pT00"] = nc.dram_tensor("d_pT00", [128, 1024], BF16,
                                     kind="ExternalOutput")
        dbg["acc_a00"] = nc.dram_tensor("d_acc_a00", [65, QC], F32,
                                        kind="ExternalOutput")
        dbg["acc_b00"] = nc.dram_tensor("d_acc_b00", [65, QC], F32,
                                        kind="ExternalOutput")
        dbg["r00"] = nc.dram_tensor("d_r00", [2, QC], F32,
                                    kind="ExternalOutput")

    with tile.TileContext(nc) as tc:
        with ExitStack() as ctx:
            wpool = ctx.enter_context(tc.tile_pool(name="wpool", bufs=1))
            xpool = ctx.enter_context(tc.tile_pool(name="xpool", bufs=6))
            qxpool = ctx.enter_context(tc.tile_pool(name="qxpool", bufs=8))
            qkpool = ctx.enter_context(tc.tile_pool(name="qkpool", bufs=1))
            v1pool = ctx.enter_context(tc.tile_pool(name="v1pool", bufs=1))
            ptpool = ctx.enter_context(tc.tile_pool(name="ptpool", bufs=3))
            nrmpool = ctx.enter_context(tc.tile_pool(name="nrmpool", bufs=2))
            pbpool = ctx.enter_context(tc.tile_pool(name="pbpool", bufs=1))
            opool = ctx.enter_context(tc.tile_pool(name="opool", bufs=3))

            t_wq = wpool.tile([128, NDC * 256], BF16, tag="wq")
            t_wk = wpool.tile([128, NDC * 256], BF16, tag="wk")
            t_wv = wpool.tile([128, NDC * 256], BF16, tag="wv")
            nc.sync.dma_start(t_wv[:], wv[:])
            nc.sync.dma_start(t_wk[:], wk[:])
            nc.sync.dma_start(t_wq[:], wq[:])
            t_wo = [wpool.tile([128, D], BF16, tag=f"wo{p}", name=f"t_wo{p}")
                    for p in range(2)]
            nc.sync.dma_start(t_wo[0][:], wo[0:128, :])
            nc.sync.dma_start(t_wo[1][:], wo[128:256, :])
            t_vm = wpool.tile([128, HPC * NKT], F32, tag="vm")
            nc.sync.dma_start(t_vm[:], vmask[:])

            # warm the ACT exp table during the projection head
            t_scr = wpool.tile([1, 1], F32, tag="scr")
            nc.vector.memset(t_scr[:], 0.0)
            t_scr2 = wpool.tile([1, 1], F32, tag="scr2")
            nc.scalar.activation(t_scr2[:], t_scr[:], AF.Exp)

            # Q^T / K^T: [128 dims (2 slots), N] per slot-pair
            t_qT = [qkpool.tile([128, N], BF16, tag=f"qT{p}", name=f"t_qT{p}")
                    for p in range(2)]
            t_kT = [qkpool.tile([128, N], BF16, tag=f"kT{p}", name=f"t_kT{p}")
                    for p in range(2)]
            # V1 per slot: trips[j] tiles of [128, 65] ([V | ones-masked])
            t_v1 = [v1pool.tile([128, 65 * trips[j]], BF16, tag=f"v1_{j}",
                                name=f"t_v1_{j}")
                    for j in range(HPC)]
            # normalized heads^T per slot pair: [128 dims, N]
            t_pb = [pbpool.tile([128, N], BF16, tag=f"pb{p}", name=f"t_pb{p}")
                    for p in range(2)]

            # ---- phase 1: V projection + K projection (8-bank PSUM pool) ----
            with tc.tile_pool(name="pp", bufs=8, space="PSUM") as pp:
                # V: two half-column passes of 8 k-tiles
                for g in range(2):
                    accs = [pp.tile([128, 256], F32, tag="acc",
                                    name=f"accv{g}_{i}") for i in range(8)]
                    for c in range(NDC):
                        xt = xpool.tile([128, 1024], BF16, tag="xt")
                        nc.sync.dma_start(
                            xt[:], xTv[c * 128:(c + 1) * 128,
                                       g * 1024:(g + 1) * 1024])
                        for kt8 in range(8):
                            nc.tensor.matmul(
                                accs[kt8][:],
                                xt[:, kt8 * 128:(kt8 + 1) * 128],
                                t_wv[:, c * 256:(c + 1) * 256],
                                start=(c == 0), stop=(c == NDC - 1))
                    for kt8 in range(8):
                        t = g * 8 + kt8
                        for j in range(HPC):
                            if t >= trips[j]:
                                continue
                            mask_col = t_vm[:, j * NKT + t: j * NKT + t + 1]
                            # V columns scaled by mask (zero rows >= vl)
                            nc.scalar.activation(
                                t_v1[j][:, t * 65: t * 65 + 64],
                                accs[kt8][:, j * 64:(j + 1) * 64],
                                AF.Copy, scale=mask_col)
                            # ones column = mask itself
                            with nc.allow_low_precision(reason="mask is 0/1"):
                                nc.vector.tensor_copy(
                                    t_v1[j][:, t * 65 + 64: t * 65 + 65],
                                    mask_col)
                # K projection: full N, 8 banks
                accs = [pp.tile([128, QC], F32, tag="acc", name=f"acck_{i}")
                        for i in range(8)]
                for c in range(NDC):
                    xt = xpool.tile([128, N], BF16, tag="xt")
                    nc.sync.dma_start(xt[:], xTk[c * 128:(c + 1) * 128, :])
                    for m in range(2):
                        for kq in range(NCH):
                            nc.tensor.matmul(
                                accs[m * NCH + kq][:],
                                t_wk[:, c * 256 + m * 128: c * 256 + (m + 1) * 128],
                                xt[:, kq * QC:(kq + 1) * QC],
                                start=(c == 0), stop=(c == NDC - 1))
                # copies ordered k-chunk-major so ladder(0) unblocks early
                for kq in range(NCH):
                    for m in range(2):
                        with nc.allow_low_precision(reason="bf16 kT"):
                            nc.vector.tensor_copy(
                                t_kT[m][:, kq * QC:(kq + 1) * QC],
                                accs[m * NCH + kq][:])

            # ---- phase 2: attention, Q-proj + out-proj software-pipelined ----
            with tc.tile_pool(name="ap", bufs=1, space="PSUM") as ap:

                def qproj_units(q):
                    """18 emission units: 8x(dma+mm m0), copy0, 8x mm m1, copy1."""
                    qs = slice(q * QC, (q + 1) * QC)
                    units = []
                    state = {"xts": [None] * NDC, "acc": None}

                    def mk_m0(c):
                        def u():
                            if state["acc"] is None:
                                state["acc"] = ap.tile(
                                    [128, QC], F32, tag="mix", bufs=2,
                                    name=f"qacc0_{q}")
                            xt = qxpool.tile([128, QC], BF16, tag="qx")
                            nc.sync.dma_start(
                                xt[:], xTq[c * 128:(c + 1) * 128, qs])
                            state["xts"][c] = xt
                            nc.tensor.matmul(
                                state["acc"][:],
                                t_wq[:, c * 256: c * 256 + 128],
                                xt[:], start=(c == 0), stop=(c == NDC - 1))
                        return u

                    def copy0():
                        with nc.allow_low_precision(reason="bf16 qT"):
                            nc.vector.tensor_copy(t_qT[0][:, qs],
                                                  state["acc"][:])
                        state["acc"] = None

                    def mk_m1(c):
                        def u():
                            if state["acc"] is None:
                                state["acc"] = ap.tile(
                                    [128, QC], F32, tag="mix", bufs=2,
                                    name=f"qacc1_{q}")
                            nc.tensor.matmul(
                                state["acc"][:],
                                t_wq[:, c * 256 + 128: c * 256 + 256],
                                state["xts"][c][:],
                                start=(c == 0), stop=(c == NDC - 1))
                        return u

                    def copy1():
                        with nc.allow_low_precision(reason="bf16 qT"):
                            nc.vector.tensor_copy(t_qT[1][:, qs],
                                                  state["acc"][:])
                        state["acc"] = None

                    for c in range(NDC):
                        units.append(mk_m0(c))
                    units.append(copy0)
                    for c in range(NDC):
                        units.append(mk_m1(c))
                    units.append(copy1)
                    return units

                def outproj_units(q):
                    """8 units: (qt, ch) matmul-pair + stage copy + dma."""
                    units = []

                    def mk(qt, ch):
                        def u():
                            ts = slice(qt * 128, (qt + 1) * 128)
                            o_ps = ap.tile([128, QC], F32, tag="mix", bufs=2,
                                           name=f"ops_{qt}_{ch}")
                            nc.tensor.matmul(
                                o_ps[:], t_pb[0][:, ts],
                                t_wo[0][:, ch * QC:(ch + 1) * QC],
                                start=True, stop=False)
                            nc.tensor.matmul(
                                o_ps[:], t_pb[1][:, ts],
                                t_wo[1][:, ch * QC:(ch + 1) * QC],
                                start=False, stop=True)
                            stg = opool.tile([128, QC], F32, tag="ostg")
                            nc.vector.tensor_copy(stg[:], o_ps[:])
                            nc.sync.dma_start(out[ts, ch * QC:(ch + 1) * QC],
                                              stg[:])
                        return u

                    for qt in range(q * (QC // 128), (q + 1) * (QC // 128)):
                        for ch in range(2):
                            units.append(mk(qt, ch))
                    return units

                def normalize_e(q, p, e, acc):
                    qs = slice(q * QC, (q + 1) * QC)
                    if DEBUG_DUMP and q == 0 and p == 0:
                        stg = opool.tile([65, QC], F32, tag="dbga")
                        nc.vector.tensor_copy(stg[:], acc[:])
                        nc.sync.dma_start(
                            dbg["acc_a00" if e == 0 else "acc_b00"][:],
                            stg[:])
                    # reciprocal_approx_fast misreads PSUM inputs on HW;
                    # stage the denominator row through SBUF first
                    den = nrmpool.tile([1, QC], F32, tag="den")
                    nc.vector.tensor_copy(den[:], acc[64:65, :])
                    r = nrmpool.tile([1, QC], F32, tag="r")
                    nc.vector.reciprocal_approx_fast(r[:], den[:])
                    if DEBUG_DUMP and q == 0 and p == 0:
                        nc.sync.dma_start(dbg["r00"][e:e + 1, :], r[:])
                    bc = nrmpool.tile([64, QC], F32, tag="bc")
                    nc.gpsimd.partition_broadcast(bc[:], r[:])
                    with nc.allow_low_precision(reason="bf16 heads"):
                        nc.vector.tensor_mul(
                            t_pb[p][e * 64:(e + 1) * 64, qs],
                            acc[0:64, :], bc[:])

                def ladder(q, p, fillers, delay=0):
                    """S/exp/PV ladder for chunk q, slot pair p.

                    Emission order per t: S_a, S_b (concurrent PE row-groups),
                    fused exp, PV_a, PV_b; fillers spread across iterations
                    starting at `delay`. Slot b's normalize is emitted as soon
                    as its accumulator stops, hiding its latency under the
                    slot-a tail; slot a's follows the loop.
                    """
                    qs = slice(q * QC, (q + 1) * QC)
                    ja, jb = 2 * p, 2 * p + 1
                    A, Bt = trips[ja], trips[jb]
                    acc_b = ap.tile([65, QC], F32, tag="acc2", bufs=2,
                                    name=f"acc_b{q}{p}")
                    acc_a = ap.tile([65, QC], F32, tag="acc2", bufs=2,
                                    name=f"acc_a{q}{p}")
                    done = 0
                    for t in range(A):
                        both = t < Bt
                        ks = slice(t * 128, (t + 1) * 128)
                        sT = ap.tile([128, 2 * QC], F32, tag="sT", bufs=2,
                                     name="sT")
                        nc.tensor.matmul(sT[:, 0:QC], t_kT[p][0:64, ks],
                                         t_qT[p][0:64, qs],
                                         start=True, stop=True)
                        if both:
                            nc.tensor.matmul(sT[:, QC:2 * QC],
                                             t_kT[p][64:128, ks],
                                             t_qT[p][64:128, qs],
                                             start=True, stop=True)
                        w = 2 * QC if both else QC
                        pT = ptpool.tile([128, 2 * QC], BF16, tag="pT")
                        with nc.allow_low_precision(reason="bf16 probs"):
                            nc.scalar.activation(pT[:, 0:w], sT[:, 0:w],
                                                 AF.Exp, scale=0.125)
                        if DEBUG_DUMP and q == 0 and p == 0 and t == 0:
                            stg = opool.tile([128, 2 * QC], F32, tag="dbgs")
                            nc.vector.tensor_copy(stg[:], sT[:])
                            nc.sync.dma_start(dbg["sT00"][:], stg[:])
                            nc.sync.dma_start(dbg["pT00"][:], pT[:])
                        nc.tensor.matmul(
                            acc_a[:], t_v1[ja][:, t * 65:(t + 1) * 65],
                            pT[:, 0:QC], start=(t == 0), stop=(t == A - 1))
                        if both:
                            nc.tensor.matmul(
                                acc_b[:], t_v1[jb][:, t * 65:(t + 1) * 65],
                                pT[:, QC:2 * QC], start=(t == 0),
                                stop=(t == Bt - 1))
                        if t == Bt - 1 and Bt < A:
                            normalize_e(q, p, 1, acc_b)
                        if t >= delay:
                            want = ((t + 1 - delay) * len(fillers)
                                    // max(A - delay, 1))
                            while done < want:
                                fillers[done]()
                                done += 1
                    if Bt == A:
                        normalize_e(q, p, 1, acc_b)
                    normalize_e(q, p, 0, acc_a)
                    while done < len(fillers):
                        fillers[done]()
                        done += 1

                # chunk 0's Q projection runs serially after K proj
                for u in qproj_units(0):
                    u()
                for q in range(NCH):
                    f0 = outproj_units(q - 1) if q > 0 else []
                    ladder(q, 0, f0, delay=3)
                    f1 = qproj_units(q + 1) if q < NCH - 1 else []
                    ladder(q, 1, f1)
                for u in outproj_units(NCH - 1):
                    u()
                if DEBUG_DUMP:
                    for p in range(2):
                        nc.sync.dma_start(dbg[f"qT{p}"][:], t_qT[p][:])
                        nc.sync.dma_start(dbg[f"kT{p}"][:], t_kT[p][:])
                        nc.sync.dma_start(dbg[f"pb{p}"][:], t_pb[p][:])
                    for j in range(HPC):
                        nc.sync.dma_start(dbg[f"v1_{j}"][:], t_v1[j][:])

    nc.finalize()
    return nc


def kernel(queries, keys, values, valid_len, Wq, Wk, Wv, Wo):
    global LAST_RESULTS
    queries = np.asarray(queries, dtype=np.float32)
    keys = np.asarray(keys, dtype=np.float32)
    values = np.asarray(values, dtype=np.float32)
    Wq = np.asarray(Wq, dtype=np.float32)
    Wk = np.asarray(Wk, dtype=np.float32)
    Wv = np.asarray(Wv, dtype=np.float32)
    Wo = np.asarray(Wo, dtype=np.float32)
    vl = np.asarray(valid_len).astype(np.int64).reshape(B * H)

    # rank-aligned slot assignment: per batch, heads sorted by vl desc;
    # slot j of the 4 cores of that batch takes ranks 4j..4j+3
    order = {}
    for b in range(B):
        idx = (np.argsort(-vl[b * H:(b + 1) * H], kind="stable") + b * H)
        for cg in range(4):
            order[b * 4 + cg] = [int(idx[4 * j + cg]) for j in range(HPC)]
    trips = []
    for j in range(HPC):
        m = max(int(-(-vl[order[c][j]] // 128)) for c in range(NCORES))
        trips.append(max(1, min(NKT, m)))

    nc = _build_program(tuple(trips))

    in_maps = []
    for c in range(NCORES):
        b = c // 4
        heads = order[c]
        cols = np.concatenate(
            [np.arange((h - b * H) * DH, (h - b * H + 1) * DH) for h in heads])

        def wlayout(w):
            return np.ascontiguousarray(
                w[:, cols].reshape(NDC, 128, 256).transpose(1, 0, 2)
                .reshape(128, NDC * 256)).astype(BFNP)

        vm = np.zeros((128, HPC * NKT), np.float32)
        for j, h in enumerate(heads):
            keep = (np.arange(N) < vl[h]).astype(np.float32)
            vm[:, j * NKT:(j + 1) * NKT] = keep.reshape(NKT, 128).T

        in_maps.append({
            "xTq": np.ascontiguousarray(queries[b].T).astype(BFNP),
            "xTk": np.ascontiguousarray(keys[b].T).astype(BFNP),
            "xTv": np.ascontiguousarray(values[b].T).astype(BFNP),
            "wq": wlayout(Wq),
            "wk": wlayout(Wk),
            "wv": wlayout(Wv),
            "wo": np.ascontiguousarray(Wo[cols, :]).astype(BFNP),
            "vmask": vm,
        })

    LAST_RESULTS = run_bass_kernel_spmd(nc, in_maps, list(range(NCORES)))
    res = LAST_RESULTS.results

    out = np.zeros((B, N, D), np.float64)
    for c in range(NCORES):
        out[c // 4] += res[c]["out"].astype(np.float64)
    return out.astype(np.float32)


# revision 16
# speedup vs baseline: 1.9539x; 1.1106x over previous
"""Multi-head attention TRN2 kernel (8 NeuronCores, SPMD).

Problem: B=2, N=2048, D=1024, H=16 heads of dim 64, fp32 in/out, per-(b,h)
key-length masking (valid_len, length 32).

Sharding: batch*heads across 8 cores - core c handles batch b=c//4 and 4
heads ("slots"), rank-aligned by valid_len so the per-slot key-tile trip
counts (uniform across cores, specialized at build time) are minimal.

v4 design (HAM-warm, ACT-paced, minimal serial head):
  - All matmul operands bf16 (host converts; PSUM accumulation stays fp32).
  - Junk matmuls at t=0 warm the HAM clock gate while weights DMA.
  - Serial head is only: V proj k-tiles 0..7, K proj chunk 0, Q proj chunk
    0. The rest of K proj and V proj ride as fillers inside the first
    attention ladder, using the spare "mix" PSUM banks; Q proj of chunk
    q+1 and out-proj of chunk q-1 ride inside later ladders.
  - S^T per slot-pair: two K=64 matmuls emitted back-to-back run
    concurrently in different PE row-groups (tile_position auto-derived).
    Both halves land in one [128,1024] 2-bank PSUM tile; ONE fused Exp
    (scale=1/8) covers both slots -> bf16 pT.
  - Softmax denominator via masked-ones column in V1.
  - Normalize: DVE reciprocal_approx_fast (SBUF-staged; the custom op
    misreads PSUM on HW) + GpSimd partition_broadcast + DVE multiply.
    Slot b normalizes as soon as its accumulator stops (hidden in ladder).
  - PSUM budget (8 banks): sT 2x2 + acc2 2 + mix 2.
"""
import sys
import numpy as np
from contextlib import ExitStack

sys.path.insert(0, "/opt/trn_rl_repo")

import ml_dtypes  # noqa: E402
import concourse.bass as bass  # noqa: E402
from concourse import bacc, mybir  # noqa: E402
import concourse.tile as tile  # noqa: E402
from concourse.bass_utils import run_bass_kernel_spmd  # noqa: E402

F32 = mybir.dt.float32
BF16 = mybir.dt.bfloat16
AF = mybir.ActivationFunctionType
MUL = mybir.AluOpType.mult
BFNP = ml_dtypes.bfloat16

B, N, D, H = 2, 2048, 1024, 16
DH = 64
HPC = 4          # heads (slots) per core
NCORES = 8
QC = 512         # q chunk (matmul free dim)
NKT = N // 128   # 16 k tiles
NDC = D // 128   # 8 contraction chunks
NCH = N // QC    # 4 q chunks

LAST_RESULTS = None  # BassKernelResults of the most recent run (for tooling)
DEBUG_DUMP = False   # add DRAM dumps of intermediates (debugging only)


def _build_program(trips):
    """trips: 4 ints - k-tile count per slot (uniform across cores)."""
    nc = bacc.Bacc("TRN2", target_bir_lowering=False, debug=False,
                   num_devices=NCORES)
    maxtr = max(trips)

    xTq = nc.dram_tensor("xTq", [D, N], BF16, kind="ExternalInput")
    xTk = nc.dram_tensor("xTk", [D, N], BF16, kind="ExternalInput")
    xTv = nc.dram_tensor("xTv", [D, N], BF16, kind="ExternalInput")
    wq = nc.dram_tensor("wq", [128, NDC * 256], BF16, kind="ExternalInput")
    wk = nc.dram_tensor("wk", [128, NDC * 256], BF16, kind="ExternalInput")
    wv = nc.dram_tensor("wv", [128, NDC * 256], BF16, kind="ExternalInput")
    wo = nc.dram_tensor("wo", [256, D], BF16, kind="ExternalInput")
    vmask = nc.dram_tensor("vmask", [128, HPC * NKT], F32, kind="ExternalInput")
    out = nc.dram_tensor("out", [N, D], F32, kind="ExternalOutput")
    dbg = {}
    if DEBUG_DUMP:
        for p in range(2):
            dbg[f"qT{p}"] = nc.dram_tensor(f"d_qT{p}", [128, N], BF16,
                                           kind="ExternalOutput")
            dbg[f"kT{p}"] = nc.dram_tensor(f"d_kT{p}", [128, N], BF16,
                                           kind="ExternalOutput")
            dbg[f"pb{p}"] = nc.dram_tensor(f"d_pb{p}", [128, N], BF16,
                                           kind="ExternalOutput")
        for j in range(HPC):
            dbg[f"v1_{j}"] = nc.dram_tensor(f"d_v1_{j}", [128, 65 * trips[j]],
                                            BF16, kind="ExternalOutput")
        dbg["sT00"] = nc.dram_tensor("d_sT00", [128, 1024], F32,
                                     kind="ExternalOutput")
        dbg["pT00"] = nc.dram_tensor("d_pT00", [128, 1024], BF16,
                                     kind="ExternalOutput")
        dbg["acc_a00"] = nc.dram_tensor("d_acc_a00", [65, QC], F32,
                                        kind="ExternalOutput")
        dbg["acc_b00"] = nc.dram_tensor("d_acc_b00", [65, QC], F32,
                                        kind="ExternalOutput")
        dbg["r00"] = nc.dram_tensor("d_r00", [2, QC], F32,
                                    kind="ExternalOutput")

    with tile.TileContext(nc) as tc:
        with ExitStack() as ctx:
            wpool = ctx.enter_context(tc.tile_pool(name="wpool", bufs=1))
            xvpool = ctx.enter_context(tc.tile_pool(name="xvpool", bufs=16))
            xkpool = ctx.enter_context(tc.tile_pool(name="xkpool", bufs=16))
            qxpool = ctx.enter_context(tc.tile_pool(name="qxpool", bufs=8))
            qkpool = ctx.enter_context(tc.tile_pool(name="qkpool", bufs=1))
            v1pool = ctx.enter_context(tc.tile_pool(name="v1pool", bufs=1))
            ptpool = ctx.enter_context(tc.tile_pool(name="ptpool", bufs=3))
            nrmpool = ctx.enter_context(tc.tile_pool(name="nrmpool", bufs=2))
            pbpool = ctx.enter_context(tc.tile_pool(name="pbpool", bufs=1))
            opool = ctx.enter_context(tc.tile_pool(name="opool", bufs=3))
            ap = ctx.enter_context(tc.tile_pool(name="ap", bufs=1,
                                                space="PSUM"))

            # ---- HAM warm-up: junk matmuls while weights stream in ----
            t_junk = wpool.tile([128, QC], BF16, tag="junk")
            nc.vector.memset(t_junk[:], 0.0)
            for i in range(14):
                wps = ap.tile([128, QC], F32, tag="mix", bufs=2,
                              name=f"warm{i}")
                nc.tensor.matmul(wps[:], t_junk[:, 0:128], t_junk[:],
                                 start=True, stop=True)

            t_wq = wpool.tile([128, NDC * 256], BF16, tag="wq")
            t_wk = wpool.tile([128, NDC * 256], BF16, tag="wk")
            t_wv = wpool.tile([128, NDC * 256], BF16, tag="wv")
            t_vm = wpool.tile([128, HPC * NKT], F32, tag="vm")
            t_wo = [wpool.tile([128, D], BF16, tag=f"wo{p}", name=f"t_wo{p}")
                    for p in range(2)]

            # warm the ACT exp table during the projection head
            t_scr = wpool.tile([1, 1], F32, tag="scr")
            nc.vector.memset(t_scr[:], 0.0)
            t_scr2 = wpool.tile([1, 1], F32, tag="scr2")
            nc.scalar.activation(t_scr2[:], t_scr[:], AF.Exp)

            # x-slice DMA caches (issued in need-order below)
            xv_tiles, xk_tiles, qx_tiles = {}, {}, {}

            def xv_tile(g, c):
                if (g, c) not in xv_tiles:
                    t = xvpool.tile([128, 1024], BF16, tag="xv")
                    nc.sync.dma_start(
                        t[:], xTv[c * 128:(c + 1) * 128,
                                  g * 1024:(g + 1) * 1024])
                    xv_tiles[(g, c)] = t
                return xv_tiles[(g, c)]

            def xk_tile(kq, c):
                if (kq, c) not in xk_tiles:
                    t = xkpool.tile([128, QC], BF16, tag="xk")
                    nc.sync.dma_start(
                        t[:], xTk[c * 128:(c + 1) * 128,
                                  kq * QC:(kq + 1) * QC])
                    xk_tiles[(kq, c)] = t
                return xk_tiles[(kq, c)]

            def qx_tile(q, c):
                if (q, c) not in qx_tiles:
                    t = qxpool.tile([128, QC], BF16, tag="qx")
                    nc.sync.dma_start(
                        t[:], xTq[c * 128:(c + 1) * 128,
                                  q * QC:(q + 1) * QC])
                    qx_tiles[(q, c)] = t
                return qx_tiles[(q, c)]

            # DMAs in need-order
            nc.sync.dma_start(t_wv[:], wv[:])
            nc.sync.dma_start(t_vm[:], vmask[:])
            for c in range(NDC):
                xv_tile(0, c)
            nc.sync.dma_start(t_wk[:], wk[:])
            for c in range(NDC):
                xk_tile(0, c)
            nc.sync.dma_start(t_wq[:], wq[:])
            for c in range(NDC):
                qx_tile(0, c)
            for c in range(NDC):
                xk_tile(1, c)
            if maxtr > 8:
                for c in range(NDC):
                    xv_tile(1, c)
            for kq in (2, 3):
                for c in range(NDC):
                    xk_tile(kq, c)
            nc.sync.dma_start(t_wo[0][:], wo[0:128, :])
            nc.sync.dma_start(t_wo[1][:], wo[128:256, :])

            # persistent SBUF tensors
            t_qT = [qkpool.tile([128, N], BF16, tag=f"qT{p}", name=f"t_qT{p}")
                    for p in range(2)]
            t_kT = [qkpool.tile([128, N], BF16, tag=f"kT{p}", name=f"t_kT{p}")
                    for p in range(2)]
            t_v1 = [v1pool.tile([128, 65 * trips[j]], BF16, tag=f"v1_{j}",
                                name=f"t_v1_{j}")
                    for j in range(HPC)]
            t_pb = [pbpool.tile([128, N], BF16, tag=f"pb{p}", name=f"t_pb{p}")
                    for p in range(2)]

            # ---- unit builders ----
            VTAGS = ("sT", "mix", "acc2")
            VBUFS = {"sT": 2, "mix": 2, "acc2": 2}

            def vproj_unit(t, tag="mix"):
                """One V k-tile: 8 accumulating matmuls + masked copies."""
                def u():
                    g, kt8 = divmod(t, 8)
                    acc = ap.tile([128, 256], F32, tag=tag, bufs=VBUFS[tag],
                                  name=f"vacc{t}")
                    for c in range(NDC):
                        xt = xv_tile(g, c)
                        nc.tensor.matmul(
                            acc[:], xt[:, kt8 * 128:(kt8 + 1) * 128],
                            t_wv[:, c * 256:(c + 1) * 256],
                            start=(c == 0), stop=(c == NDC - 1))
                    for j in range(HPC):
                        if t >= trips[j]:
                            continue
                        mask_col = t_vm[:, j * NKT + t: j * NKT + t + 1]
                        with nc.allow_low_precision(reason="bf16 V"):
                            nc.vector.tensor_scalar(
                                t_v1[j][:, t * 65: t * 65 + 64],
                                acc[:, j * 64:(j + 1) * 64],
                                mask_col, None, MUL)
                            nc.vector.tensor_copy(
                                t_v1[j][:, t * 65 + 64: t * 65 + 65],
                                mask_col)
                return u

            def kproj_unit(kq, m):
                """K projection m-half of one 512-col chunk."""
                def u():
                    acc = ap.tile([128, QC], F32, tag="mix", bufs=2,
                                  name=f"kacc{kq}_{m}")
                    for c in range(NDC):
                        nc.tensor.matmul(
                            acc[:],
                            t_wk[:, c * 256 + m * 128: c * 256 + (m + 1) * 128],
                            xk_tile(kq, c)[:],
                            start=(c == 0), stop=(c == NDC - 1))
                    with nc.allow_low_precision(reason="bf16 kT"):
                        nc.vector.tensor_copy(
                            t_kT[m][:, kq * QC:(kq + 1) * QC], acc[:])
                return u

            def qproj_unit(q, m):
                """Q projection m-half of chunk q."""
                def u():
                    acc = ap.tile([128, QC], F32, tag="mix", bufs=2,
                                  name=f"qacc{q}_{m}")
                    for c in range(NDC):
                        nc.tensor.matmul(
                            acc[:],
                            t_wq[:, c * 256 + m * 128: c * 256 + (m + 1) * 128],
                            qx_tile(q, c)[:],
                            start=(c == 0), stop=(c == NDC - 1))
                    with nc.allow_low_precision(reason="bf16 qT"):
                        nc.vector.tensor_copy(
                            t_qT[m][:, q * QC:(q + 1) * QC], acc[:])
                return u

            def outproj_units(q, alt_tags=False):
                """8 units: (qt, ch) matmul-pair + stage copy + dma."""
                units = []

                def mk(qt, ch, tag):
                    def u():
                        ts = slice(qt * 128, (qt + 1) * 128)
                        o_ps = ap.tile([128, QC], F32, tag=tag, bufs=2,
                                       name=f"ops_{qt}_{ch}")
                        nc.tensor.matmul(
                            o_ps[:], t_pb[0][:, ts],
                            t_wo[0][:, ch * QC:(ch + 1) * QC],
                            start=True, stop=False)
                        nc.tensor.matmul(
                            o_ps[:], t_pb[1][:, ts],
                            t_wo[1][:, ch * QC:(ch + 1) * QC],
                            start=False, stop=True)
                        stg = opool.tile([128, QC], F32, tag="ostg")
                        nc.vector.tensor_copy(stg[:], o_ps[:])
                        nc.sync.dma_start(out[ts, ch * QC:(ch + 1) * QC],
                                          stg[:])
                    return u

                i = 0
                for qt in range(q * (QC // 128), (q + 1) * (QC // 128)):
                    for ch in range(2):
                        tag = ("mix", "acc2")[i % 2] if alt_tags else "mix"
                        units.append(mk(qt, ch, tag))
                        i += 1
                return units

            def normalize_e(q, p, e, acc):
                qs = slice(q * QC, (q + 1) * QC)
                if DEBUG_DUMP and q == 0 and p == 0:
                    stg = opool.tile([65, QC], F32, tag="dbga")
                    nc.vector.tensor_copy(stg[:], acc[:])
                    nc.sync.dma_start(
                        dbg["acc_a00" if e == 0 else "acc_b00"][:], stg[:])
                # reciprocal_approx_fast misreads PSUM inputs on HW;
                # stage the denominator row through SBUF first
                den = nrmpool.tile([1, QC], F32, tag="den")
                nc.vector.tensor_copy(den[:], acc[64:65, :])
                r = nrmpool.tile([1, QC], F32, tag="r")
                nc.vector.reciprocal_approx_fast(r[:], den[:])
                if DEBUG_DUMP and q == 0 and p == 0:
                    nc.sync.dma_start(dbg["r00"][e:e + 1, :], r[:])
                bc = nrmpool.tile([64, QC], F32, tag="bc")
                nc.gpsimd.partition_broadcast(bc[:], r[:])
                with nc.allow_low_precision(reason="bf16 heads"):
                    nc.vector.tensor_mul(
                        t_pb[p][e * 64:(e + 1) * 64, qs],
                        acc[0:64, :], bc[:])

            def ladder(q, p, fillers, delay=0):
                """S/exp/PV ladder for chunk q, slot pair p.

                Per t: S_a, S_b (concurrent PE row-groups), fused exp,
                PV_a, PV_b. `fillers` is a list of (fn, deadline) pairs;
                a unit with deadline d MUST be emitted before iteration d's
                ladder ops (it is emitted at the top of iteration d-2 at
                the latest); deadline None spreads evenly from `delay`.
                Slot b normalizes when its accumulator stops (hidden under
                the slot-a tail); slot a right after the loop.
                """
                qs = slice(q * QC, (q + 1) * QC)
                ja, jb = 2 * p, 2 * p + 1
                A, Bt = trips[ja], trips[jb]
                acc_b = ap.tile([65, QC], F32, tag="acc2", bufs=2,
                                name=f"acc_b{q}{p}")
                acc_a = ap.tile([65, QC], F32, tag="acc2", bufs=2,
                                name=f"acc_a{q}{p}")
                pending = list(fillers)
                n_total = len(pending)
                n_done = 0
                for t in range(A):
                    # deadline fillers first (2-iteration lookahead)
                    rest = []
                    for fn, dl in pending:
                        if dl is not None and dl <= t + 2:
                            fn()
                            n_done += 1
                        else:
                            rest.append((fn, dl))
                    pending = rest
                    both = t < Bt
                    ks = slice(t * 128, (t + 1) * 128)
                    sT = ap.tile([128, 2 * QC], F32, tag="sT", bufs=2,
                                 name="sT")
                    nc.tensor.matmul(sT[:, 0:QC], t_kT[p][0:64, ks],
                                     t_qT[p][0:64, qs],
                                     start=True, stop=True)
                    if both:
                        nc.tensor.matmul(sT[:, QC:2 * QC],
                                         t_kT[p][64:128, ks],
                                         t_qT[p][64:128, qs],
                                         start=True, stop=True)
                    w = 2 * QC if both else QC
                    pT = ptpool.tile([128, 2 * QC], BF16, tag="pT")
                    with nc.allow_low_precision(reason="bf16 probs"):
                        nc.scalar.activation(pT[:, 0:w], sT[:, 0:w],
                                             AF.Exp, scale=0.125)
                    if DEBUG_DUMP and q == 0 and p == 0 and t == 0:
                        stg = opool.tile([128, 2 * QC], F32, tag="dbgs")
                        nc.vector.tensor_copy(stg[:], sT[:])
                        nc.sync.dma_start(dbg["sT00"][:], stg[:])
                        nc.sync.dma_start(dbg["pT00"][:], pT[:])
                    nc.tensor.matmul(
                        acc_a[:], t_v1[ja][:, t * 65:(t + 1) * 65],
                        pT[:, 0:QC], start=(t == 0), stop=(t == A - 1))
                    if both:
                        nc.tensor.matmul(
                            acc_b[:], t_v1[jb][:, t * 65:(t + 1) * 65],
                            pT[:, QC:2 * QC], start=(t == 0),
                            stop=(t == Bt - 1))
                    if t == Bt - 1 and Bt < A:
                        normalize_e(q, p, 1, acc_b)
                    if t >= delay:
                        want = ((t + 1 - delay) * n_total
                                // max(A - delay, 1))
                        while pending and n_done < want:
                            fn, _ = pending.pop(0)
                            fn()
                            n_done += 1
                if Bt == A:
                    normalize_e(q, p, 1, acc_b)
                normalize_e(q, p, 0, acc_a)
                for fn, _ in pending:
                    fn()

            # ---- serial head: V k-tiles 0..7, K chunk 0, Q chunk 0 ----
            for t in range(min(8, maxtr)):
                vproj_unit(t, tag=VTAGS[t % 3])()
            kproj_unit(0, 0)()
            kproj_unit(0, 1)()
            qproj_unit(0, 0)()
            qproj_unit(0, 1)()

            # fillers for ladder(0,0): rest of K proj + V k-tiles 8..
            # deadlines: K chunk kq's m0 feeds S at iteration 4*kq; V tile t
            # feeds PV at iteration t; K m1 halves are only needed by
            # ladder(0,1) and spread freely.
            f00 = []
            vrest = [(vproj_unit(t), t) for t in range(8, maxtr)]
            for kq in (1, 2, 3):
                f00.append((kproj_unit(kq, 0), 4 * kq))
                if vrest:
                    f00.append(vrest.pop(0))
                f00.append((kproj_unit(kq, 1), None))
                if vrest:
                    f00.append(vrest.pop(0))
            f00.extend(vrest)

            for q in range(NCH):
                if q == 0:
                    ladder(q, 0, f00)
                else:
                    ladder(q, 0, [(u, None) for u in outproj_units(q - 1)],
                           delay=3)
                f1 = ([(qproj_unit(q + 1, 0), None),
                       (qproj_unit(q + 1, 1), None)]
                      if q < NCH - 1 else [])
                if q < NCH - 1:
                    for c in range(NDC):
                        qx_tile(q + 1, c)  # prefetch
                ladder(q, 1, f1)
            for u in outproj_units(NCH - 1, alt_tags=True):
                u()

            if DEBUG_DUMP:
                for p in range(2):
                    nc.sync.dma_start(dbg[f"qT{p}"][:], t_qT[p][:])
                    nc.sync.dma_start(dbg[f"kT{p}"][:], t_kT[p][:])
                    nc.sync.dma_start(dbg[f"pb{p}"][:], t_pb[p][:])
                for j in range(HPC):
                    nc.sync.dma_start(dbg[f"v1_{j}"][:], t_v1[j][:])

    nc.finalize()
    return nc


def kernel(queries, keys, values, valid_len, Wq, Wk, Wv, Wo):
    global LAST_RESULTS
    queries = np.asarray(queries, dtype=np.float32)
    keys = np.asarray(keys, dtype=np.float32)
    values = np.asarray(values, dtype=np.float32)
    Wq = np.asarray(Wq, dtype=np.float32)
    Wk = np.asarray(Wk, dtype=np.float32)
    Wv = np.asarray(Wv, dtype=np.float32)
    Wo = np.asarray(Wo, dtype=np.float32)
    vl = np.asarray(valid_len).astype(np.int64).reshape(B * H)

    # rank-aligned slot assignment: per batch, heads sorted by vl desc;
    # slot j of the 4 cores of that batch takes ranks 4j..4j+3
    order = {}
    for b in range(B):
        idx = (np.argsort(-vl[b * H:(b + 1) * H], kind="stable") + b * H)
        for cg in range(4):
            order[b * 4 + cg] = [int(idx[4 * j + cg]) for j in range(HPC)]
    trips = []
    for j in range(HPC):
        m = max(int(-(-vl[order[c][j]] // 128)) for c in range(NCORES))
        trips.append(max(1, min(NKT, m)))

    nc = _build_program(tuple(trips))

    in_maps = []
    for c in range(NCORES):
        b = c // 4
        heads = order[c]
        cols = np.concatenate(
            [np.arange((h - b * H) * DH, (h - b * H + 1) * DH) for h in heads])

        def wlayout(w):
            return np.ascontiguousarray(
                w[:, cols].reshape(NDC, 128, 256).transpose(1, 0, 2)
                .reshape(128, NDC * 256)).astype(BFNP)

        vm = np.zeros((128, HPC * NKT), np.float32)
        for j, h in enumerate(heads):
            keep = (np.arange(N) < vl[h]).astype(np.float32)
            vm[:, j * NKT:(j + 1) * NKT] = keep.reshape(NKT, 128).T

        in_maps.append({
            "xTq": np.ascontiguousarray(queries[b].T).astype(BFNP),
            "xTk": np.ascontiguousarray(keys[b].T).astype(BFNP),
            "xTv": np.ascontiguousarray(values[b].T).astype(BFNP),
            "wq": wlayout(Wq),
            "wk": wlayout(Wk),
            "wv": wlayout(Wv),
            "wo": np.ascontiguousarray(Wo[cols, :]).astype(BFNP),
            "vmask": vm,
        })

    LAST_RESULTS = run_bass_kernel_spmd(nc, in_maps, list(range(NCORES)))
    res = LAST_RESULTS.results

    out = np.zeros((B, N, D), np.float64)
    for c in range(NCORES):
        out[c // 4] += res[c]["out"].astype(np.float64)
    return out.astype(np.float32)
